# revision 41
# baseline (speedup 1.0000x reference)
"""Multi-head attention (B=2, S=2048, D=1024, H=16) on 8 Trainium2 cores.

Sharding: core i -> batch i//4, head-group i%4 (4 heads = 2 pairs of 2).

v4.6: softmax exp on ACT is the per-core floor (~138us busy); the whole
schedule keeps ACT busy on exp from t~11us with the PE's in-order
stream never blocking the score matmuls that feed it:
 - Phase 1 holds only what must precede block 2's scores: warmup,
   qproj(sb0), key-chunk-granular kproj interleaved with blocks 0/1
   scores/exp (Q-major block order), qproj(sb1) pair-0 half.
 - A pre-burst of v-proj chunks 0-9 fills the PE-idle window while
   exp(b1) finishes; v-proj 10-15 and the remaining q-proj halves are
   deadline-ordered "extras" woven one per slot through later blocks,
   ping-ponging through a 2-buf misc PSUM pool (no rotation stalls).
 - attnV runs in [q-partition, dim-free] layout (lhsT = P, rhs = V-chunk
   + ones column -> normalizer Z in column 64), drained from a work
   queue behind the scores; per-block accumulators are freed by one
   bulk DVE copy, with normalization/PE-transpose reading SBUF.
 - Output projection contracts BOTH head-pairs into one PSUM tile ->
   a single [2048,1024] bf16 partial per core.
 - ACT does exp only until the final block, whose tail splits DVE/ACT.
Host sums 8 bf16 partials and adds bv@Wo + bo.
"""

import sys

import numpy as np

try:
    import concourse.bacc as bacc
except ImportError:  # grading dir may not have the repo on sys.path
    sys.path.insert(0, "/opt/trn_rl_repo")
    import concourse.bacc as bacc

import ml_dtypes
import concourse.mybir as mybir
import concourse.tile as tile
from concourse import bass_utils

B, S, D, H, DH = 2, 2048, 1024, 16, 64
F32 = mybir.dt.float32
R32 = mybir.dt.float32r
BF16 = mybir.dt.bfloat16
EXP = mybir.ActivationFunctionType.Exp
COPY = mybir.ActivationFunctionType.Copy


def _emit(nc, aps):
    xq, xk, xv = aps["xqT"], aps["xkT"], aps["xvT"]
    out_ap = aps["out"]

    with tile.TileContext(nc) as tc, \
         nc.allow_low_precision(reason="bf16 x/w/P pipeline"), \
         tc.tile_pool(name="persist", bufs=1, space="SBUF") as sb, \
         tc.tile_pool(name="xstream", bufs=5, space="SBUF") as xp, \
         tc.tile_pool(name="pexp", bufs=40, space="SBUF") as pa_pool, \
         tc.tile_pool(name="avs", bufs=2, space="SBUF") as avs_pool, \
         tc.tile_pool(name="anorm", bufs=8, space="SBUF") as an_pool, \
         tc.tile_pool(name="atrans", bufs=36, space="SBUF") as at_pool, \
         tc.tile_pool(name="obuf", bufs=2, space="SBUF") as ob_pool, \
         tc.tile_pool(name="rpool", bufs=2, space="SBUF") as rp_pool, \
         tc.tile_pool(name="sp", bufs=2, space="PSUM") as sp:

        wq_sb = sb.tile([128, 2048], BF16)
        wk_sb = sb.tile([128, 2048], BF16)
        wv_sb = sb.tile([128, 2048], BF16)
        wo_sb = sb.tile([128, 2048], BF16)
        bqT_sb = sb.tile([128, 2], F32)
        bkT_sb = sb.tile([128, 2], F32)
        ident_sb = sb.tile([128, 128], BF16)
        qT_sb = sb.tile([128, 4096], R32)
        kT_sb = sb.tile([128, 4096], R32)
        # per key-chunk j (260 cols): [p0h0 v(64) 1 | p0h1 v(64) 1 | p1...]
        vaug_sb = sb.tile([128, 16 * 260], BF16)
        dum_sb = sb.tile([128, 512], BF16)

        z512_sb = sb.tile([128, 512], BF16)
        nc.vector.memset(dum_sb[:], 1.0)
        nc.vector.memset(z512_sb[:], 0.0)
        vj = vaug_sb[:].rearrange("p (g c) -> p g c", c=65)
        nc.vector.memset(vj[:, :, 64:65], 1.0)

        def wdma(dst, src):
            nc.sync.dma_start(dst[:].rearrange("p (d c) -> p d c", c=256),
                              src[:].rearrange("(d p) c -> p d c", p=128))

        def wdma_cc(dst, src, cc):
            # wq/wk dram is pre-packed [p, cc*1024 + d*128 + c] so each
            # cc half is one contiguous 256KB DMA (2KB lines, 128 descs)
            nc.sync.dma_start(dst[:, cc * 1024:(cc + 1) * 1024],
                              src[:, cc * 1024:(cc + 1) * 1024])

        wdma_cc(wq_sb, aps["wq"], 0)

        # PE clock ramp: dummy matmuls until the first x piece lands;
        # sized to abut it so the ramp never resets
        wup = sp.tile([128, 1024], F32, tag="s", name="wup")
        for i in range(14):
            nc.tensor.matmul(wup[:, 0:256], dum_sb[:, 0:128],
                             dum_sb[:, 0:256], start=True, stop=True)

        def stream_slab(src, sbi, nm, pieces=1):
            xt = xp.tile([128, 4096], BF16, tag="xs", name=nm)
            dstv = xt[:].rearrange("p (d c) -> p d c", c=512)
            srcv = src[:, sbi * 512:(sbi + 1) * 512] \
                .rearrange("(d p) c -> p d c", p=128)
            if pieces == 1:
                nc.sync.dma_start(dstv, srcv)
            else:
                dn = 8 // pieces
                for pc in range(pieces):
                    nc.sync.dma_start(dstv[:, pc * dn:(pc + 1) * dn, :],
                                      srcv[:, pc * dn:(pc + 1) * dn, :])
            return xt

        def stream_slab_h2(src, sbi, nm):
            # two seq-half DMAs: finer exp gating without hitting the
            # per-descriptor DMA floor (256B lines would double the cost)
            xt = xp.tile([128, 4096], BF16, tag="xs", name=nm)
            dstv = xt[:].rearrange("p (d c) -> p d c", c=512)
            srcv = src[:, sbi * 512:(sbi + 1) * 512] \
                .rearrange("(d p) c -> p d c", p=128)
            for hh in range(2):
                nc.sync.dma_start(dstv[:, :, hh * 256:(hh + 1) * 256],
                                  srcv[:, :, hh * 256:(hh + 1) * 256])
            return xt

        def proj_q_cc(pool, xt, sbi, cc, tag):
            acc = pool.tile([128, 512], F32, tag=tag, name=f"q{sbi}_{cc}")
            for dc in range(8):
                nc.tensor.matmul(
                    acc[:],
                    wq_sb[:, cc * 1024 + dc * 128:cc * 1024 + dc * 128 + 128],
                    xt[:, dc * 512:(dc + 1) * 512],
                    start=(dc == 0), stop=(dc == 7))
            nc.vector.tensor_scalar_add(
                qT_sb[:, cc * 2048 + sbi * 512:cc * 2048 + sbi * 512 + 512],
                acc[:], bqT_sb[:, cc:cc + 1])

        # Q-major block order: blocks 0/1 only need q seq-block 0.
        blocks = [(p, Q * 512, 512) for Q in range(4) for p in range(2)]
        blocks = blocks[:-1] + [(1, 1536, 256), (1, 1792, 256)]
        nbl = len(blocks)
        pBigs = {}   # (bi, j) -> exp'd scores [128 keys, 2 heads x qlen] bf16
        queue = []   # attnV chunks (bi, j) not yet emitted
        sBigs = {}   # (bi, j) -> PSUM score tile awaiting exp

        def scores_mm(bi, j):
            p, qoff, qlen = blocks[bi]
            qb = p * 2048 + qoff
            kb = p * 2048 + j * 128
            sBig = sp.tile([128, 1024], F32, tag="s", name=f"s{bi}_{j}")
            nc.tensor.matmul(sBig[:, 0:qlen],
                             kT_sb[0:64, kb:kb + 128],
                             qT_sb[0:64, qb:qb + qlen],
                             start=True, stop=True)
            nc.tensor.matmul(sBig[:, 512:512 + qlen],
                             kT_sb[64:128, kb:kb + 128],
                             qT_sb[64:128, qb:qb + qlen],
                             start=True, stop=True)
            sBigs[(bi, j)] = sBig

        def exp_act(bi, j):
            p, qoff, qlen = blocks[bi]
            sBig = sBigs.pop((bi, j))
            pb = pa_pool.tile([128, 1024], BF16, tag="pa", name=f"pb{bi}_{j}")
            if qlen == 512:
                nc.scalar.activation(pb[:], sBig[:], EXP, scale=0.125)
            else:
                sv = sBig[:].rearrange("p (g c) -> p g c", c=512)[:, :, 0:qlen]
                pv = pb[:, 0:2 * qlen].rearrange("p (g c) -> p g c", c=qlen)
                nc.scalar.activation(pv, sv, EXP, scale=0.125)
            pBigs[(bi, j)] = pb
            queue.append((bi, j))

        def scores_exp(bi, j):
            scores_mm(bi, j)
            exp_act(bi, j)

        # ---- phase 1: only what must precede block 2's scores ----
        # DMA order (HWDGE costs ~650ns per DMA serially and transfers
        # serialize at ~22.5B/ns, so few, big DMAs win): wq, xq0 (2
        # pieces), wk, bq, bk, xk0-3 (each seq-halved), wv, xq1, xv0-3,
        # xq2, xq3, ident, wo. cc-grouped start: block-0 scores need only
        # the cc0 halves of qT/kT, so the cc0 path runs first.
        xv_tiles = {}
        xq_tiles = {}
        with tc.tile_pool(name="pp1", bufs=2, space="PSUM") as pp1:
            xt = stream_slab(xq, 0, "xq0", pieces=2)
            wdma_cc(wk_sb, aps["wk"], 0)
            nc.sync.dma_start(bqT_sb[:], aps["bqT"][:])
            nc.sync.dma_start(bkT_sb[:], aps["bkT"][:])
            xk0 = stream_slab_h2(xk, 0, "xk0")
            wdma_cc(wq_sb, aps["wq"], 1)
            wdma_cc(wk_sb, aps["wk"], 1)

            def proj_cc(w_sb, xt, cc, acc):
                for dc in range(8):
                    nc.tensor.matmul(
                        acc[:],
                        w_sb[:, cc * 1024 + dc * 128:cc * 1024 + dc * 128 + 128],
                        xt[:, dc * 512:(dc + 1) * 512],
                        start=(dc == 0), stop=(dc == 7))

            def kproj_chunk(jj, cc):
                ka = pp1.tile([128, 128], F32, tag="kc",
                              name=f"k0c{jj}_{cc}", bufs=2)
                for dc in range(8):
                    nc.tensor.matmul(
                        ka[:],
                        wk_sb[:, cc * 1024 + dc * 128:cc * 1024 + dc * 128 + 128],
                        xk0[:, dc * 512 + jj * 128:dc * 512 + jj * 128 + 128],
                        start=(dc == 0), stop=(dc == 7))
                c0 = cc * 2048 + jj * 128
                nc.vector.tensor_scalar_add(
                    kT_sb[:, c0:c0 + 128], ka[:], bkT_sb[:, cc:cc + 1])

            # cc0 path -> block 0 scores flow as each k chunk lands
            q0acc = pp1.tile([128, 512], F32, tag="pp", name="q0_0")
            proj_cc(wq_sb, xt, 0, q0acc)
            nc.vector.tensor_scalar_add(
                qT_sb[:, 0:512], q0acc[:], bqT_sb[:, 0:1])
            for jj in range(4):
                kproj_chunk(jj, 0)
                scores_exp(0, jj)
            # cc1 path -> block 1 scores
            q0acc1 = pp1.tile([128, 512], F32, tag="pp", name="q0_1")
            proj_cc(wq_sb, xt, 1, q0acc1)
            nc.vector.tensor_scalar_add(
                qT_sb[:, 2048:2048 + 512], q0acc1[:], bqT_sb[:, 1:2])
            for jj in range(4):
                kproj_chunk(jj, 1)
                scores_exp(1, jj)
            for sbi in range(1, 4):
                xt = stream_slab_h2(xk, sbi, f"xk{sbi}")
                for hh in range(2):
                    kaccs = [pp1.tile([128, 256], F32, tag="pp",
                                      name=f"k{sbi}_{hh}_{cc}")
                             for cc in range(2)]
                    for dc in range(8):
                        for cc in range(2):
                            nc.tensor.matmul(
                                kaccs[cc][:],
                                wk_sb[:, cc * 1024 + dc * 128:cc * 1024 + dc * 128 + 128],
                                xt[:, dc * 512 + hh * 256:dc * 512 + hh * 256 + 256],
                                start=(dc == 0), stop=(dc == 7))
                    for cc in range(2):
                        c0 = cc * 2048 + sbi * 512 + hh * 256
                        nc.vector.tensor_scalar_add(
                            kT_sb[:, c0:c0 + 256], kaccs[cc][:],
                            bkT_sb[:, cc:cc + 1])
                    for jj in range(2):
                        scores_exp(0, sbi * 4 + hh * 2 + jj)
                        scores_exp(1, sbi * 4 + hh * 2 + jj)
            wdma(wv_sb, aps["wv"])
            xq_tiles[1] = stream_slab(xq, 1, "xq1")
            for sbi in range(4):
                xv_tiles[sbi] = stream_slab(xv, sbi, f"xv{sbi}")
            for sbi in range(2, 4):
                xq_tiles[sbi] = stream_slab(xq, sbi, f"xq{sbi}")
            nc.sync.dma_start(ident_sb[:], aps["ident"][:])
            nc.sync.dma_start(wo_sb[:].rearrange("p (g c) -> p g c", c=1024),
                              aps["wo"][:].rearrange("(g p) c -> p g c", p=128))
            # block 2 = (pair0, Q1) needs qT sb1's cc0 half before its
            # first scores; cc1 (block 3) runs as a slot-12/13 pending
            # burst to keep the phase-1 tail (the handoff gap) short
            proj_q_cc(pp1, xq_tiles[1], 1, 0, "pp")

        # ---- attention + normalization + output projection ----
        with tc.tile_pool(name="av", bufs=1, space="PSUM") as av_pool, \
             tc.tile_pool(name="ms", bufs=2, space="PSUM") as ms_pool:

            av_tiles = {}
            avs_tiles = {}
            state = {}     # bi -> {"ans": [...], "ats": [...]}
            pending = {}   # slot -> [thunk, ...]
            slot = [0]
            vp_done = [0]  # v chunks emitted (gate for the attnV drain)
            ex_state = {}

            def vp_chunk(j, half=None):
                sbi, jj = j // 4, j % 4
                xt = xv_tiles[sbi]
                halves = (0, 1) if half is None else (half,)
                if halves[0] == 0:
                    ex_state[j] = ms_pool.tile([128, 512], F32, tag="ms",
                                               name=f"vpx{j}")
                vps = ex_state[j]
                for hf in halves:
                    for dc in range(hf * 4, hf * 4 + 4):
                        nc.tensor.matmul(
                            vps[:, 0:256],
                            xt[:, dc * 512 + jj * 128:dc * 512 + jj * 128 + 128],
                            wv_sb[:, dc * 256:(dc + 1) * 256],
                            start=(dc == 0), stop=(dc == 7))
                if halves[-1] == 1:
                    dst = vaug_sb[:, j * 260:j * 260 + 260] \
                        .rearrange("p (g c) -> p g c", c=65)[:, :, 0:64]
                    nc.vector.tensor_copy(
                        dst,
                        vps[:, 0:256].rearrange("p (g c) -> p g c", c=64))
                    del ex_state[j]
                    vp_done[0] = j + 1

            def qp_thunk(sbi, cc, dc):
                if dc == 0:
                    ex_state["qp"] = ms_pool.tile([128, 512], F32, tag="ms",
                                                  name=f"qpx{sbi}_{cc}")
                acc = ex_state["qp"]
                nc.tensor.matmul(
                    acc[:],
                    wq_sb[:, cc * 1024 + dc * 128:cc * 1024 + dc * 128 + 128],
                    xq_tiles[sbi][:, dc * 512:(dc + 1) * 512],
                    start=(dc == 0), stop=(dc == 7))
                if dc == 7:
                    nc.vector.tensor_scalar_add(
                        qT_sb[:, cc * 2048 + sbi * 512:cc * 2048 + sbi * 512 + 512],
                        acc[:], bqT_sb[:, cc:cc + 1])

            def qp_burst(sbi, cc, dcs):
                for dc in dcs:
                    qp_thunk(sbi, cc, dc)

            # (pre-burst is emitted after the first two b2 score chunks,
            # below, so ACT has queued work while the PE runs it)

            # extras: v-proj halves early (drain gating); the late q-proj
            # slabs go in as paired 4-dc pending bursts at their LATEST
            # safe slots so the congested mid-phase stays clear. Burst B
            # is the first pending of its slot so the shared ms tile is
            # consumed before any tp/up allocation rotates onto it.
            pe_extras = []
            for j in range(6, 16):
                for half in range(2):
                    pe_extras.append(lambda j=j, half=half: vp_chunk(j, half))
            qp_sched = {(1, 1): 12, (2, 0): 29, (2, 1): 45,
                        (3, 0): 61, (3, 1): 77}

            rts = {}

            def norm(bi):
                p, qoff, qlen = blocks[bi]
                nqc = qlen // 128
                last = bi == nbl - 1
                avs_t = avs_tiles[bi]
                rt = rts[bi]
                ans = []
                for qc in range(nqc):
                    an = an_pool.tile([128, 128], BF16, tag="an",
                                      name=f"an{bi}_{qc}")
                    for h in range(2):
                        k = qc * 2 + h
                        if last and h == 1:
                            # ACT is idle after the final exp
                            nc.scalar.activation(
                                an[:, h * 64:(h + 1) * 64],
                                avs_t[:, k * 128:k * 128 + 64],
                                COPY, scale=rt[:, k:k + 1])
                        else:
                            nc.vector.tensor_scalar_mul(
                                an[:, h * 64:(h + 1) * 64],
                                avs_t[:, k * 128:k * 128 + 64],
                                rt[:, k:k + 1])
                    ans.append(an)
                state[bi] = {"ans": ans}

            def trans(bi):
                last = bi == nbl - 1
                ats = []
                for qc, an in enumerate(state[bi]["ans"]):
                    tpt = ms_pool.tile([128, 128], BF16, tag="ms",
                                       name=f"tp{bi}_{qc}")
                    nc.tensor.transpose(tpt[:, 0:128], an[:], ident_sb[:])
                    at = at_pool.tile([128, 128], BF16, tag="at",
                                      name=f"at{bi}_{qc}")
                    if last and qc % 2 == 1:
                        nc.scalar.copy(at[:], tpt[:, 0:128])
                    else:
                        nc.vector.tensor_copy(at[:], tpt[:, 0:128])
                    ats.append(at)
                state[bi]["ats"] = ats

            def outproj_piece(Q, qc, n):
                # contract BOTH head-pairs (2 x 128 dims) into one tile
                last = Q == 3
                if Q < 3:
                    at0 = state[2 * Q]["ats"][qc]
                    at1 = state[2 * Q + 1]["ats"][qc]
                else:
                    at0 = state[6]["ats"][qc]
                    at1 = state[7 if qc < 2 else 8]["ats"][qc % 2]
                key = (Q, qc)
                if n == 0:
                    state.setdefault("ob", {})[key] = ob_pool.tile(
                        [128, 1024], BF16, tag="ob", name=f"ob{Q}_{qc}")
                obt = state["ob"][key]
                # the tail pieces use the score ring (free after the last
                # exp) so they don't ping-pong the 2-buf ms ring with the
                # final transposes
                tailp = last and qc >= 2
                pool, tag = (sp, "s") if tailp else (ms_pool, "ms")
                up = pool.tile([128, 512], F32, tag=tag,
                               name=f"up{Q}_{qc}_{n}")
                nc.tensor.matmul(up[:], at0[:],
                                 wo_sb[:, n * 512:n * 512 + 512],
                                 start=True, stop=False)
                nc.tensor.matmul(up[:], at1[:],
                                 wo_sb[:, 1024 + n * 512:1024 + n * 512 + 512],
                                 start=False, stop=True)
                if last and n == 1 and qc >= 2:
                    nc.scalar.copy(obt[:, n * 512:(n + 1) * 512], up[:])
                else:
                    nc.vector.tensor_copy(obt[:, n * 512:(n + 1) * 512],
                                          up[:])
                if n == 1:
                    nc.sync.dma_start(
                        out_ap[Q * 512 + qc * 128:Q * 512 + qc * 128 + 128, :],
                        obt[:])

            def attnv_guard(av_t, nbank, start):
                # PSUM accumulation groups are 2KB-bank granular and the
                # scheduler may reorder disjoint-column matmuls, so each
                # bank's group is bracketed by full-bank zero matmuls:
                # start overwrites the bank with zeros (uniform pending-
                # zero + WAW edge to every slot), stop accumulates zeros
                # (RAW/WAW edge from every slot, closes the group)
                for b4 in range(nbank):
                    if start:
                        nc.tensor.matmul(
                            av_t[:, b4 * 512:(b4 + 1) * 512],
                            dum_sb[:, 0:128], z512_sb[:],
                            start=True, stop=False)
                    else:
                        # close: tiny strided zero-add overlapping each
                        # slot's first column (pending-zero already clear)
                        gv = av_t[:].rearrange("p (s c) -> p s c", c=128)
                        nc.tensor.matmul(
                            gv[:, b4 * 4:b4 * 4 + 4, 0:1],
                            dum_sb[:, 0:128], z512_sb[:, 0:4],
                            start=False, stop=True)

            def attnv(bi, j):
                p, qoff, qlen = blocks[bi]
                pb = pBigs[(bi, j)]
                av_t = av_tiles[bi]
                nqc = qlen // 128
                if j == 0:
                    attnv_guard(av_t, (2 * nqc + 3) // 4, True)
                for qc in range(nqc):
                    for h in range(2):
                        k = qc * 2 + h
                        nc.tensor.matmul(
                            av_t[:, k * 128:k * 128 + 65],
                            pb[:, h * qlen + qc * 128:h * qlen + qc * 128 + 128],
                            vaug_sb[:, j * 260 + p * 130 + h * 65:
                                    j * 260 + p * 130 + h * 65 + 65],
                            start=False, stop=False)
                if j == 15:
                    attnv_guard(av_t, (2 * nqc + 3) // 4, False)

            def crossing(b):
                # free block b-1's PSUM accumulator with one bulk copy,
                # then schedule its normalization/transpose; the output
                # projection fires once BOTH pairs of a Q are transposed
                s = slot[0]
                if b > 0:
                    pb_, qoff_, qlen_ = blocks[b - 1]
                    nqc = qlen_ // 128
                    avs_t = avs_pool.tile([128, 1024], F32, tag="avs",
                                          name=f"avs{b - 1}")
                    # strided: only the written 65-col slots — shorter
                    # copy on the accumulator-rotation critical path (the
                    # full-bank start guards own the gap bytes)
                    sv = av_tiles[b - 1][:].rearrange(
                        "p (s c) -> p s c", c=128)[:, 0:2 * nqc, 0:65]
                    dv = avs_t[:].rearrange(
                        "p (s c) -> p s c", c=128)[:, 0:2 * nqc, 0:65]
                    nc.vector.tensor_copy(dv, sv)
                    avs_tiles[b - 1] = avs_t
                    # reciprocal straight from the PSUM Z columns: runs in
                    # parallel with the bulk copy, off the norm critical
                    # path (matters for the final block's tail chain)
                    rt = rp_pool.tile([128, 8], F32, tag="r",
                                      name=f"rt{b - 1}")
                    zv = av_tiles[b - 1][:].rearrange(
                        "p (s c) -> p s c", c=128)[:, 0:2 * nqc, 64:65] \
                        .rearrange("p s c -> p (s c)")
                    nc.vector.reciprocal(rt[:, 0:2 * nqc], zv)
                    rts[b - 1] = rt
                    pending.setdefault(s + 1, []).append(
                        lambda b=b: norm(b - 1))
                    pending.setdefault(s + 3, []).append(
                        lambda b=b: trans(b - 1))
                    if b - 1 in (1, 3, 5):
                        # defer outproj pieces into the PE-idle regions:
                        # Q0/Q1 into the blocks-5/6 surplus, Q2 into the
                        # fast 256-block region (ACT there only needs
                        # ~0.37us/slot of PE work), spread 1-per-2-slots
                        Q = (b - 1) // 2
                        base = max(s + 5, {0: 48, 1: 64, 2: 76}[Q])
                        pieces = [(qc, n) for qc in range(4)
                                  for n in range(2)]
                        for i, (qc, n) in enumerate(pieces):
                            pending.setdefault(base + 2 * i, []).append(
                                lambda Q=Q, qc=qc, n=n:
                                outproj_piece(Q, qc, n))
                    elif b - 1 == 7:
                        # Q3's first half reads blocks 6+7 only — fire
                        # as soon as block 7 is transposed
                        for i, (qc, n) in enumerate(
                                [(qc, n) for qc in range(2)
                                 for n in range(2)]):
                            pending.setdefault(s + 5 + i, []).append(
                                lambda qc=qc, n=n: outproj_piece(3, qc, n))
                if b < nbl:
                    av_tiles[b] = av_pool.tile([128, 1024], F32, tag="av",
                                               name=f"av{b}")

            order = [(b, c) for b in range(nbl) for c in range(16)]
            qptr = [0]

            def drain(cap):
                # block-major: a block's 16 chunks fully accumulate (and
                # its crossing copy fires) before the next block starts
                crossed = False
                for _ in range(cap):
                    if qptr[0] >= len(order):
                        break
                    b, c = order[qptr[0]]
                    if (b, c) not in pBigs or c >= vp_done[0]:
                        break
                    if c == 0:
                        if crossed:
                            break
                        crossing(b)
                        crossed = True
                    qptr[0] += 1
                    attnv(b, c)

            # exp spine for the main loop; scores are emitted one slot
            # AHEAD of their exp so the next exp never sits behind the
            # current slot's drain/extras in the PE stream
            spine = [(bi, j) for bi in range(2, nbl) for j in range(16)]

            def run_slot(k=None, cap=2):
                if k is not None:
                    if k + 1 < len(spine):
                        scores_mm(*spine[k + 1])
                    exp_act(*spine[k])
                for th in pending.pop(slot[0], []):
                    th()
                drain(3 if len(pBigs) - qptr[0] > 28 else cap)
                if pe_extras:
                    pe_extras.pop(0)()
                slot[0] += 1

            for (sbi, cc), s in qp_sched.items():
                pending.setdefault(s, []).append(
                    lambda sbi=sbi, cc=cc: qp_burst(sbi, cc, range(0, 4)))
                pending.setdefault(s + 1, []).insert(
                    0, lambda sbi=sbi, cc=cc: qp_burst(sbi, cc, range(4, 8)))

            # first b2 score chunk, then the v-proj pre-burst fills the
            # PE-idle window while exp(b1)'s tail runs
            scores_mm(2, 0)
            for j in range(6):
                vp_chunk(j)
            for k in range(len(spine)):
                run_slot(k)
            # tail: drain remaining attnV chunks + flush pending work
            while qptr[0] < len(order) or pe_extras or pending:
                run_slot(cap=3)
            crossing(nbl)  # schedules the final block's norm/trans
            while pending:
                run_slot(cap=3)
            # final output projection (Q3 second half: blocks 6 + 8)
            for qc in range(2, 4):
                for n in range(2):
                    outproj_piece(3, qc, n)


_NC = None


def _get_nc():
    global _NC
    if _NC is None:
        nc = bacc.Bacc("TRN2", target_bir_lowering=False, debug=False,
                       enable_asserts=False, num_devices=8)
        aps = {}
        for nm, shp in [("xqT", (D, S)), ("xkT", (D, S)), ("xvT", (D, S)),
                        ("wq", (128, 2048)), ("wk", (128, 2048)),
                        ("wv", (D, 256))]:
            aps[nm] = nc.dram_tensor(nm, shp, BF16, kind="ExternalInput").ap()
        aps["ident"] = nc.dram_tensor("ident", (128, 128), BF16,
                                      kind="ExternalInput").ap()
        aps["wo"] = nc.dram_tensor("wo", (256, D), BF16, kind="ExternalInput").ap()
        for nm, shp in [("bqT", (128, 2)), ("bkT", (128, 2))]:
            aps[nm] = nc.dram_tensor(nm, shp, F32, kind="ExternalInput").ap()
        aps["out"] = nc.dram_tensor("out", (S, D), BF16,
                                    kind="ExternalOutput").ap()
        _emit(nc, aps)
        nc.compile()
        _NC = nc
    return _NC


def _run(inputs, trace=False):
    nc = _get_nc()
    f = np.float32
    bf = ml_dtypes.bfloat16
    q = np.asarray(inputs["query"], dtype=f)
    k = np.asarray(inputs["key"], dtype=f)
    v = np.asarray(inputs["value"], dtype=f)
    Wq = np.asarray(inputs["Wq"], dtype=f)
    Wk = np.asarray(inputs["Wk"], dtype=f)
    Wv = np.asarray(inputs["Wv"], dtype=f)
    Wo = np.asarray(inputs["Wo"], dtype=f)
    bq = np.asarray(inputs["bq"], dtype=f)
    bk = np.asarray(inputs["bk"], dtype=f)
    bv = np.asarray(inputs["bv"], dtype=f)
    bo = np.asarray(inputs["bo"], dtype=f)

    ident = np.eye(128, dtype=f)
    xT = {b: (np.ascontiguousarray(q[b].T).astype(bf),
              np.ascontiguousarray(k[b].T).astype(bf),
              np.ascontiguousarray(v[b].T).astype(bf)) for b in range(B)}
    def pack_w(W):
        # (1024, 256) -> (128, 2048) laid out [p, cc*1024 + d*128 + c]
        Wr = W.reshape(8, 128, 2, 128)
        return np.ascontiguousarray(
            Wr.transpose(1, 2, 0, 3).reshape(128, 2048)).astype(bf)

    in_maps = []
    for i in range(8):
        b, hg = divmod(i, 4)
        c0 = hg * 256
        in_maps.append({
            "xqT": xT[b][0], "xkT": xT[b][1], "xvT": xT[b][2],
            "wq": pack_w(Wq[:, c0:c0 + 256]),
            "wk": pack_w(Wk[:, c0:c0 + 256]),
            "wv": np.ascontiguousarray(Wv[:, c0:c0 + 256]).astype(bf),
            "bqT": np.ascontiguousarray(bq[c0:c0 + 256].reshape(2, 128).T),
            "bkT": np.ascontiguousarray(bk[c0:c0 + 256].reshape(2, 128).T),
            "wo": np.ascontiguousarray(Wo[c0:c0 + 256, :]).astype(bf),
            "ident": ident.astype(bf),
        })

    res = bass_utils.run_bass_kernel_spmd(nc, in_maps, core_ids=list(range(8)),
                                          trace=trace)
    out = np.zeros((B, S, D), dtype=f)
    for i in range(8):
        out[i // 4] += np.asarray(res.results[i]["out"], dtype=f)
    out += (bv @ Wo + bo)[None, None, :]
    return out, res


def kernel(**inputs):
    out, _ = _run(inputs, trace=False)
    return out



# revision 44
# speedup vs baseline: 1.0051x; 1.0051x over previous
"""Multi-head attention (B=2, S=2048, D=1024, H=16) on 8 Trainium2 cores.

Sharding: core i -> batch i//4, head-group i%4 (4 heads = 2 pairs of 2).

v4.8 (~172us, from 177.6): PE busy (~146us) and ACT exp (~136us) are
both near their floors, so the schedule load-balances the PE stream
against the fixed 144-exp ACT spine:
 - scores are emitted one slot AHEAD of their exp (2-buf score ring)
   so the next exp never sits behind the current slot's drain/extras.
 - phase 1 is DMA-order-optimized (HWDGE costs ~650ns per DMA and the
   transfer queue is serial): wq/wk are host-packed [p, cc*1024+d*128+c]
   so each cc half is one contiguous 256KB DMA, and block-0 scores only
   need the cc0 halves -> first exp at ~10.3us. k-slabs stream in
   seq-halves with half-granular k-proj feeding the phase-1 spine.
 - v-proj runs as full-chunk extras in the first slots (the ACT backlog
   left by phase 1 absorbs the overload); late q-proj slabs are paired
   4-dc pending bursts at their LATEST safe slots; outproj Q0/Q1/Q2
   pieces are deferred into the measured PE-idle regions (1-per-2-slots)
   instead of firing at block crossings.
 - attnV in [q-partition, dim-free] layout (lhsT = P, rhs = V-chunk +
   ones column -> Z in column 64); per-block PSUM accumulators freed by
   one bulk DVE copy; normalization on DVE, transposes in bf16 (an/at/
   ident/tpt) so they run 1 cyc/row and the at-copies hit DVE 2x mode.
 - output projection contracts BOTH head-pairs into one PSUM tile (wo
   in bf16 to keep matmul dtypes legal); the final Q3 qc2/3 pieces use
   the freed score ring instead of ping-ponging the ms ring, and every
   obt row ships as a single full-row DMA.
Host sums 8 bf16 partials and adds bv@Wo + bo.
"""

import sys

import numpy as np

try:
    import concourse.bacc as bacc
except ImportError:  # grading dir may not have the repo on sys.path
    sys.path.insert(0, "/opt/trn_rl_repo")
    import concourse.bacc as bacc

import ml_dtypes
import concourse.mybir as mybir
import concourse.tile as tile
from concourse import bass_utils

B, S, D, H, DH = 2, 2048, 1024, 16, 64
F32 = mybir.dt.float32
R32 = mybir.dt.float32r
BF16 = mybir.dt.bfloat16
EXP = mybir.ActivationFunctionType.Exp
COPY = mybir.ActivationFunctionType.Copy


def _emit(nc, aps):
    xq, xk, xv = aps["xqT"], aps["xkT"], aps["xvT"]
    out_ap = aps["out"]

    with tile.TileContext(nc) as tc, \
         nc.allow_low_precision(reason="bf16 x/w/P pipeline"), \
         tc.tile_pool(name="persist", bufs=1, space="SBUF") as sb, \
         tc.tile_pool(name="xstream", bufs=5, space="SBUF") as xp, \
         tc.tile_pool(name="pexp", bufs=40, space="SBUF") as pa_pool, \
         tc.tile_pool(name="avs", bufs=2, space="SBUF") as avs_pool, \
         tc.tile_pool(name="anorm", bufs=8, space="SBUF") as an_pool, \
         tc.tile_pool(name="atrans", bufs=36, space="SBUF") as at_pool, \
         tc.tile_pool(name="obuf", bufs=2, space="SBUF") as ob_pool, \
         tc.tile_pool(name="rpool", bufs=2, space="SBUF") as rp_pool, \
         tc.tile_pool(name="sp", bufs=2, space="PSUM") as sp:

        wq_sb = sb.tile([128, 2048], BF16)
        wk_sb = sb.tile([128, 2048], BF16)
        wv_sb = sb.tile([128, 2048], BF16)
        wo_sb = sb.tile([128, 2048], BF16)
        bqT_sb = sb.tile([128, 2], F32)
        bkT_sb = sb.tile([128, 2], F32)
        ident_sb = sb.tile([128, 128], BF16)
        qT_sb = sb.tile([128, 4096], R32)
        kT_sb = sb.tile([128, 4096], R32)
        # per key-chunk j (260 cols): [p0h0 v(64) 1 | p0h1 v(64) 1 | p1...]
        vaug_sb = sb.tile([128, 16 * 260], BF16)
        dum_sb = sb.tile([128, 512], BF16)

        z512_sb = sb.tile([128, 512], BF16)
        nc.vector.memset(dum_sb[:], 1.0)
        nc.vector.memset(z512_sb[:], 0.0)
        vj = vaug_sb[:].rearrange("p (g c) -> p g c", c=65)
        nc.vector.memset(vj[:, :, 64:65], 1.0)

        def wdma(dst, src):
            nc.sync.dma_start(dst[:].rearrange("p (d c) -> p d c", c=256),
                              src[:].rearrange("(d p) c -> p d c", p=128))

        def wdma_cc(dst, src, cc):
            # wq/wk dram is pre-packed [p, cc*1024 + d*128 + c] so each
            # cc half is one contiguous 256KB DMA (2KB lines, 128 descs)
            nc.sync.dma_start(dst[:, cc * 1024:(cc + 1) * 1024],
                              src[:, cc * 1024:(cc + 1) * 1024])

        wdma_cc(wq_sb, aps["wq"], 0)

        # PE clock ramp: dummy matmuls until the first x piece lands;
        # sized to abut it so the ramp never resets
        wup = sp.tile([128, 1024], F32, tag="s", name="wup")
        for i in range(14):
            nc.tensor.matmul(wup[:, 0:256], dum_sb[:, 0:128],
                             dum_sb[:, 0:256], start=True, stop=True)

        def stream_slab(src, sbi, nm, pieces=1):
            xt = xp.tile([128, 4096], BF16, tag="xs", name=nm)
            dstv = xt[:].rearrange("p (d c) -> p d c", c=512)
            srcv = src[:, sbi * 512:(sbi + 1) * 512] \
                .rearrange("(d p) c -> p d c", p=128)
            if pieces == 1:
                nc.sync.dma_start(dstv, srcv)
            else:
                dn = 8 // pieces
                for pc in range(pieces):
                    nc.sync.dma_start(dstv[:, pc * dn:(pc + 1) * dn, :],
                                      srcv[:, pc * dn:(pc + 1) * dn, :])
            return xt

        def stream_slab_h2(src, sbi, nm):
            # two seq-half DMAs: finer exp gating without hitting the
            # per-descriptor DMA floor (256B lines would double the cost)
            xt = xp.tile([128, 4096], BF16, tag="xs", name=nm)
            dstv = xt[:].rearrange("p (d c) -> p d c", c=512)
            srcv = src[:, sbi * 512:(sbi + 1) * 512] \
                .rearrange("(d p) c -> p d c", p=128)
            for hh in range(2):
                nc.sync.dma_start(dstv[:, :, hh * 256:(hh + 1) * 256],
                                  srcv[:, :, hh * 256:(hh + 1) * 256])
            return xt

        def proj_q_cc(pool, xt, sbi, cc, tag):
            acc = pool.tile([128, 512], F32, tag=tag, name=f"q{sbi}_{cc}")
            for dc in range(8):
                nc.tensor.matmul(
                    acc[:],
                    wq_sb[:, cc * 1024 + dc * 128:cc * 1024 + dc * 128 + 128],
                    xt[:, dc * 512:(dc + 1) * 512],
                    start=(dc == 0), stop=(dc == 7))
            nc.vector.tensor_scalar_add(
                qT_sb[:, cc * 2048 + sbi * 512:cc * 2048 + sbi * 512 + 512],
                acc[:], bqT_sb[:, cc:cc + 1])

        # Q-major block order: blocks 0/1 only need q seq-block 0.
        blocks = [(p, Q * 512, 512) for Q in range(4) for p in range(2)]
        blocks = blocks[:-1] + [(1, 1536, 256), (1, 1792, 256)]
        nbl = len(blocks)
        pBigs = {}   # (bi, j) -> exp'd scores [128 keys, 2 heads x qlen] bf16
        queue = []   # attnV chunks (bi, j) not yet emitted
        sBigs = {}   # (bi, j) -> PSUM score tile awaiting exp

        def scores_mm(bi, j):
            p, qoff, qlen = blocks[bi]
            qb = p * 2048 + qoff
            kb = p * 2048 + j * 128
            sBig = sp.tile([128, 1024], F32, tag="s", name=f"s{bi}_{j}")
            nc.tensor.matmul(sBig[:, 0:qlen],
                             kT_sb[0:64, kb:kb + 128],
                             qT_sb[0:64, qb:qb + qlen],
                             start=True, stop=True)
            nc.tensor.matmul(sBig[:, 512:512 + qlen],
                             kT_sb[64:128, kb:kb + 128],
                             qT_sb[64:128, qb:qb + qlen],
                             start=True, stop=True)
            sBigs[(bi, j)] = sBig

        def exp_act(bi, j):
            p, qoff, qlen = blocks[bi]
            sBig = sBigs.pop((bi, j))
            pb = pa_pool.tile([128, 1024], BF16, tag="pa", name=f"pb{bi}_{j}")
            if qlen == 512:
                nc.scalar.activation(pb[:], sBig[:], EXP, scale=0.125)
            else:
                sv = sBig[:].rearrange("p (g c) -> p g c", c=512)[:, :, 0:qlen]
                pv = pb[:, 0:2 * qlen].rearrange("p (g c) -> p g c", c=qlen)
                nc.scalar.activation(pv, sv, EXP, scale=0.125)
            pBigs[(bi, j)] = pb
            queue.append((bi, j))

        def scores_exp(bi, j):
            scores_mm(bi, j)
            exp_act(bi, j)

        # ---- phase 1: only what must precede block 2's scores ----
        # DMA order (HWDGE costs ~650ns per DMA serially and transfers
        # serialize at ~22.5B/ns, so few, big DMAs win): wq, xq0 (2
        # pieces), wk, bq, bk, xk0-3 (each seq-halved), wv, xq1, xv0-3,
        # xq2, xq3, ident, wo. cc-grouped start: block-0 scores need only
        # the cc0 halves of qT/kT, so the cc0 path runs first.
        xv_tiles = {}
        xq_tiles = {}
        with tc.tile_pool(name="pp1", bufs=2, space="PSUM") as pp1:
            xt = stream_slab(xq, 0, "xq0", pieces=2)
            wdma_cc(wk_sb, aps["wk"], 0)
            nc.sync.dma_start(bqT_sb[:], aps["bqT"][:])
            nc.sync.dma_start(bkT_sb[:], aps["bkT"][:])
            xk0 = stream_slab_h2(xk, 0, "xk0")
            wdma_cc(wq_sb, aps["wq"], 1)
            wdma_cc(wk_sb, aps["wk"], 1)

            def proj_cc(w_sb, xt, cc, acc):
                for dc in range(8):
                    nc.tensor.matmul(
                        acc[:],
                        w_sb[:, cc * 1024 + dc * 128:cc * 1024 + dc * 128 + 128],
                        xt[:, dc * 512:(dc + 1) * 512],
                        start=(dc == 0), stop=(dc == 7))

            def kproj_chunk(jj, cc):
                ka = pp1.tile([128, 128], F32, tag="kc",
                              name=f"k0c{jj}_{cc}", bufs=2)
                for dc in range(8):
                    nc.tensor.matmul(
                        ka[:],
                        wk_sb[:, cc * 1024 + dc * 128:cc * 1024 + dc * 128 + 128],
                        xk0[:, dc * 512 + jj * 128:dc * 512 + jj * 128 + 128],
                        start=(dc == 0), stop=(dc == 7))
                c0 = cc * 2048 + jj * 128
                nc.vector.tensor_scalar_add(
                    kT_sb[:, c0:c0 + 128], ka[:], bkT_sb[:, cc:cc + 1])

            # cc0 path -> block 0 scores flow as each k chunk lands
            q0acc = pp1.tile([128, 512], F32, tag="pp", name="q0_0")
            proj_cc(wq_sb, xt, 0, q0acc)
            nc.vector.tensor_scalar_add(
                qT_sb[:, 0:512], q0acc[:], bqT_sb[:, 0:1])
            for jj in range(4):
                kproj_chunk(jj, 0)
                scores_exp(0, jj)
            # cc1 path -> block 1 scores
            q0acc1 = pp1.tile([128, 512], F32, tag="pp", name="q0_1")
            proj_cc(wq_sb, xt, 1, q0acc1)
            nc.vector.tensor_scalar_add(
                qT_sb[:, 2048:2048 + 512], q0acc1[:], bqT_sb[:, 1:2])
            for jj in range(4):
                kproj_chunk(jj, 1)
                scores_exp(1, jj)
            for sbi in range(1, 4):
                xt = stream_slab_h2(xk, sbi, f"xk{sbi}")
                for hh in range(2):
                    kaccs = [pp1.tile([128, 256], F32, tag="pp",
                                      name=f"k{sbi}_{hh}_{cc}")
                             for cc in range(2)]
                    for dc in range(8):
                        for cc in range(2):
                            nc.tensor.matmul(
                                kaccs[cc][:],
                                wk_sb[:, cc * 1024 + dc * 128:cc * 1024 + dc * 128 + 128],
                                xt[:, dc * 512 + hh * 256:dc * 512 + hh * 256 + 256],
                                start=(dc == 0), stop=(dc == 7))
                    for cc in range(2):
                        c0 = cc * 2048 + sbi * 512 + hh * 256
                        nc.vector.tensor_scalar_add(
                            kT_sb[:, c0:c0 + 256], kaccs[cc][:],
                            bkT_sb[:, cc:cc + 1])
                    for jj in range(2):
                        scores_exp(0, sbi * 4 + hh * 2 + jj)
                        scores_exp(1, sbi * 4 + hh * 2 + jj)
            wdma(wv_sb, aps["wv"])
            xq_tiles[1] = stream_slab(xq, 1, "xq1")
            for sbi in range(4):
                xv_tiles[sbi] = stream_slab(xv, sbi, f"xv{sbi}")
            for sbi in range(2, 4):
                xq_tiles[sbi] = stream_slab(xq, sbi, f"xq{sbi}")
            nc.sync.dma_start(ident_sb[:], aps["ident"][:])
            nc.sync.dma_start(wo_sb[:].rearrange("p (g c) -> p g c", c=1024),
                              aps["wo"][:].rearrange("(g p) c -> p g c", p=128))
            # block 2 = (pair0, Q1) needs qT sb1's cc0 half before its
            # first scores; cc1 (block 3) runs as a slot-12/13 pending
            # burst to keep the phase-1 tail (the handoff gap) short
            proj_q_cc(pp1, xq_tiles[1], 1, 0, "pp")

        # ---- attention + normalization + output projection ----
        with tc.tile_pool(name="av", bufs=1, space="PSUM") as av_pool, \
             tc.tile_pool(name="ms", bufs=2, space="PSUM") as ms_pool:

            av_tiles = {}
            avs_tiles = {}
            state = {}     # bi -> {"ans": [...], "ats": [...]}
            pending = {}   # slot -> [thunk, ...]
            slot = [0]
            vp_done = [0]  # v chunks emitted (gate for the attnV drain)
            ex_state = {}

            def vp_chunk(j, half=None):
                sbi, jj = j // 4, j % 4
                xt = xv_tiles[sbi]
                halves = (0, 1) if half is None else (half,)
                if halves[0] == 0:
                    ex_state[j] = ms_pool.tile([128, 512], F32, tag="ms",
                                               name=f"vpx{j}")
                vps = ex_state[j]
                for hf in halves:
                    for dc in range(hf * 4, hf * 4 + 4):
                        nc.tensor.matmul(
                            vps[:, 0:256],
                            xt[:, dc * 512 + jj * 128:dc * 512 + jj * 128 + 128],
                            wv_sb[:, dc * 256:(dc + 1) * 256],
                            start=(dc == 0), stop=(dc == 7))
                if halves[-1] == 1:
                    dst = vaug_sb[:, j * 260:j * 260 + 260] \
                        .rearrange("p (g c) -> p g c", c=65)[:, :, 0:64]
                    nc.vector.tensor_copy(
                        dst,
                        vps[:, 0:256].rearrange("p (g c) -> p g c", c=64))
                    del ex_state[j]
                    vp_done[0] = j + 1

            def qp_thunk(sbi, cc, dc):
                if dc == 0:
                    ex_state["qp"] = ms_pool.tile([128, 512], F32, tag="ms",
                                                  name=f"qpx{sbi}_{cc}")
                acc = ex_state["qp"]
                nc.tensor.matmul(
                    acc[:],
                    wq_sb[:, cc * 1024 + dc * 128:cc * 1024 + dc * 128 + 128],
                    xq_tiles[sbi][:, dc * 512:(dc + 1) * 512],
                    start=(dc == 0), stop=(dc == 7))
                if dc == 7:
                    nc.vector.tensor_scalar_add(
                        qT_sb[:, cc * 2048 + sbi * 512:cc * 2048 + sbi * 512 + 512],
                        acc[:], bqT_sb[:, cc:cc + 1])

            def qp_burst(sbi, cc, dcs):
                for dc in dcs:
                    qp_thunk(sbi, cc, dc)

            # (pre-burst is emitted after the first two b2 score chunks,
            # below, so ACT has queued work while the PE runs it)

            # extras: v-proj halves early (drain gating); the late q-proj
            # slabs go in as paired 4-dc pending bursts at their LATEST
            # safe slots so the congested mid-phase stays clear. Burst B
            # is the first pending of its slot so the shared ms tile is
            # consumed before any tp/up allocation rotates onto it.
            pe_extras = []
            for j in range(6, 16):
                for half in range(2):
                    pe_extras.append(lambda j=j, half=half: vp_chunk(j, half))
            qp_sched = {(1, 1): 12, (2, 0): 29, (2, 1): 45,
                        (3, 0): 61, (3, 1): 77}

            def norm(bi):
                p, qoff, qlen = blocks[bi]
                nqc = qlen // 128
                last = bi == nbl - 1
                avs_t = avs_tiles[bi]
                rt = rp_pool.tile([128, 8], F32, tag="r", name=f"rt{bi}")
                zv = avs_t[:].rearrange("p (s c) -> p s c", c=128) \
                    [:, 0:2 * nqc, 64:65].rearrange("p s c -> p (s c)")
                nc.vector.reciprocal(rt[:, 0:2 * nqc], zv)
                ans = []
                for qc in range(nqc):
                    an = an_pool.tile([128, 128], BF16, tag="an",
                                      name=f"an{bi}_{qc}")
                    for h in range(2):
                        k = qc * 2 + h
                        if last and h == 1:
                            # ACT is idle after the final exp
                            nc.scalar.activation(
                                an[:, h * 64:(h + 1) * 64],
                                avs_t[:, k * 128:k * 128 + 64],
                                COPY, scale=rt[:, k:k + 1])
                        else:
                            nc.vector.tensor_scalar_mul(
                                an[:, h * 64:(h + 1) * 64],
                                avs_t[:, k * 128:k * 128 + 64],
                                rt[:, k:k + 1])
                    ans.append(an)
                state[bi] = {"ans": ans}

            def trans(bi):
                last = bi == nbl - 1
                ats = []
                for qc, an in enumerate(state[bi]["ans"]):
                    tpt = ms_pool.tile([128, 128], BF16, tag="ms",
                                       name=f"tp{bi}_{qc}")
                    nc.tensor.transpose(tpt[:, 0:128], an[:], ident_sb[:])
                    at = at_pool.tile([128, 128], BF16, tag="at",
                                      name=f"at{bi}_{qc}")
                    if last and qc % 2 == 1:
                        nc.scalar.copy(at[:], tpt[:, 0:128])
                    else:
                        nc.vector.tensor_copy(at[:], tpt[:, 0:128])
                    ats.append(at)
                state[bi]["ats"] = ats

            def outproj_piece(Q, qc, n):
                # contract BOTH head-pairs (2 x 128 dims) into one tile
                last = Q == 3
                if Q < 3:
                    at0 = state[2 * Q]["ats"][qc]
                    at1 = state[2 * Q + 1]["ats"][qc]
                else:
                    at0 = state[6]["ats"][qc]
                    at1 = state[7 if qc < 2 else 8]["ats"][qc % 2]
                key = (Q, qc)
                if n == 0:
                    state.setdefault("ob", {})[key] = ob_pool.tile(
                        [128, 1024], BF16, tag="ob", name=f"ob{Q}_{qc}")
                obt = state["ob"][key]
                # the tail pieces use the score ring (free after the last
                # exp) so they don't ping-pong the 2-buf ms ring with the
                # final transposes
                tailp = last and qc >= 2
                pool, tag = (sp, "s") if tailp else (ms_pool, "ms")
                up = pool.tile([128, 512], F32, tag=tag,
                               name=f"up{Q}_{qc}_{n}")
                nc.tensor.matmul(up[:], at0[:],
                                 wo_sb[:, n * 512:n * 512 + 512],
                                 start=True, stop=False)
                nc.tensor.matmul(up[:], at1[:],
                                 wo_sb[:, 1024 + n * 512:1024 + n * 512 + 512],
                                 start=False, stop=True)
                if last and n == 1 and qc >= 2:
                    nc.scalar.copy(obt[:, n * 512:(n + 1) * 512], up[:])
                else:
                    nc.vector.tensor_copy(obt[:, n * 512:(n + 1) * 512],
                                          up[:])
                if n == 1:
                    nc.sync.dma_start(
                        out_ap[Q * 512 + qc * 128:Q * 512 + qc * 128 + 128, :],
                        obt[:])

            def attnv_guard(av_t, nbank, start):
                # PSUM accumulation groups are 2KB-bank granular and the
                # scheduler may reorder disjoint-column matmuls, so each
                # bank's group is bracketed by full-bank zero matmuls:
                # start overwrites the bank with zeros (uniform pending-
                # zero + WAW edge to every slot), stop accumulates zeros
                # (RAW/WAW edge from every slot, closes the group)
                for b4 in range(nbank):
                    if start:
                        nc.tensor.matmul(
                            av_t[:, b4 * 512:(b4 + 1) * 512],
                            dum_sb[:, 0:128], z512_sb[:],
                            start=True, stop=False)
                    else:
                        # close: tiny strided zero-add overlapping each
                        # slot's first column (pending-zero already clear)
                        gv = av_t[:].rearrange("p (s c) -> p s c", c=128)
                        nc.tensor.matmul(
                            gv[:, b4 * 4:b4 * 4 + 4, 0:1],
                            dum_sb[:, 0:128], z512_sb[:, 0:4],
                            start=False, stop=True)

            def attnv(bi, j):
                p, qoff, qlen = blocks[bi]
                pb = pBigs[(bi, j)]
                av_t = av_tiles[bi]
                nqc = qlen // 128
                if j == 0:
                    attnv_guard(av_t, (2 * nqc + 3) // 4, True)
                for qc in range(nqc):
                    for h in range(2):
                        k = qc * 2 + h
                        nc.tensor.matmul(
                            av_t[:, k * 128:k * 128 + 65],
                            pb[:, h * qlen + qc * 128:h * qlen + qc * 128 + 128],
                            vaug_sb[:, j * 260 + p * 130 + h * 65:
                                    j * 260 + p * 130 + h * 65 + 65],
                            start=False, stop=False)
                if j == 15:
                    attnv_guard(av_t, (2 * nqc + 3) // 4, False)

            def crossing(b):
                # free block b-1's PSUM accumulator with one bulk copy,
                # then schedule its normalization/transpose; the output
                # projection fires once BOTH pairs of a Q are transposed
                s = slot[0]
                if b > 0:
                    pb_, qoff_, qlen_ = blocks[b - 1]
                    nqc = qlen_ // 128
                    avs_t = avs_pool.tile([128, 1024], F32, tag="avs",
                                          name=f"avs{b - 1}")
                    # strided: only the written 65-col slots — shorter
                    # copy on the accumulator-rotation critical path (the
                    # full-bank start guards own the gap bytes)
                    sv = av_tiles[b - 1][:].rearrange(
                        "p (s c) -> p s c", c=128)[:, 0:2 * nqc, 0:65]
                    dv = avs_t[:].rearrange(
                        "p (s c) -> p s c", c=128)[:, 0:2 * nqc, 0:65]
                    nc.vector.tensor_copy(dv, sv)
                    avs_tiles[b - 1] = avs_t
                    pending.setdefault(s + 1, []).append(
                        lambda b=b: norm(b - 1))
                    pending.setdefault(s + 3, []).append(
                        lambda b=b: trans(b - 1))
                    if b - 1 in (1, 3, 5):
                        # defer outproj pieces into the PE-idle regions:
                        # Q0/Q1 into the blocks-5/6 surplus, Q2 into the
                        # fast 256-block region (ACT there only needs
                        # ~0.37us/slot of PE work), spread 1-per-2-slots
                        Q = (b - 1) // 2
                        base = max(s + 5, {0: 48, 1: 64, 2: 76}[Q])
                        pieces = [(qc, n) for qc in range(4)
                                  for n in range(2)]
                        for i, (qc, n) in enumerate(pieces):
                            pending.setdefault(base + 2 * i, []).append(
                                lambda Q=Q, qc=qc, n=n:
                                outproj_piece(Q, qc, n))
                    elif b - 1 == 7:
                        # Q3's first half reads blocks 6+7 only — fire
                        # as soon as block 7 is transposed
                        for i, (qc, n) in enumerate(
                                [(qc, n) for qc in range(2)
                                 for n in range(2)]):
                            pending.setdefault(s + 5 + i, []).append(
                                lambda qc=qc, n=n: outproj_piece(3, qc, n))
                if b < nbl:
                    av_tiles[b] = av_pool.tile([128, 1024], F32, tag="av",
                                               name=f"av{b}")

            order = [(b, c) for b in range(nbl) for c in range(16)]
            qptr = [0]

            def drain(cap):
                # block-major: a block's 16 chunks fully accumulate (and
                # its crossing copy fires) before the next block starts
                crossed = False
                for _ in range(cap):
                    if qptr[0] >= len(order):
                        break
                    b, c = order[qptr[0]]
                    if (b, c) not in pBigs or c >= vp_done[0]:
                        break
                    if c == 0:
                        if crossed:
                            break
                        crossing(b)
                        crossed = True
                    qptr[0] += 1
                    attnv(b, c)

            # exp spine for the main loop; scores are emitted one slot
            # AHEAD of their exp so the next exp never sits behind the
            # current slot's drain/extras in the PE stream
            spine = [(bi, j) for bi in range(2, nbl) for j in range(16)]

            def run_slot(k=None, cap=2):
                if k is not None:
                    if k + 1 < len(spine):
                        scores_mm(*spine[k + 1])
                    exp_act(*spine[k])
                for th in pending.pop(slot[0], []):
                    th()
                drain(3 if len(pBigs) - qptr[0] > 28 else cap)
                if pe_extras:
                    pe_extras.pop(0)()
                slot[0] += 1

            for (sbi, cc), s in qp_sched.items():
                pending.setdefault(s, []).append(
                    lambda sbi=sbi, cc=cc: qp_burst(sbi, cc, range(0, 4)))
                pending.setdefault(s + 1, []).insert(
                    0, lambda sbi=sbi, cc=cc: qp_burst(sbi, cc, range(4, 8)))

            # first b2 score chunk, then the v-proj pre-burst fills the
            # PE-idle window while exp(b1)'s tail runs
            scores_mm(2, 0)
            for j in range(6):
                vp_chunk(j)
            for k in range(len(spine)):
                run_slot(k)
            # tail: drain remaining attnV chunks + flush pending work
            while qptr[0] < len(order) or pe_extras or pending:
                run_slot(cap=3)
            crossing(nbl)  # schedules the final block's norm/trans
            while pending:
                run_slot(cap=3)
            # final output projection (Q3 second half: blocks 6 + 8)
            for qc in range(2, 4):
                for n in range(2):
                    outproj_piece(3, qc, n)


_NC = None


def _get_nc():
    global _NC
    if _NC is None:
        nc = bacc.Bacc("TRN2", target_bir_lowering=False, debug=False,
                       enable_asserts=False, num_devices=8)
        aps = {}
        for nm, shp in [("xqT", (D, S)), ("xkT", (D, S)), ("xvT", (D, S)),
                        ("wq", (128, 2048)), ("wk", (128, 2048)),
                        ("wv", (D, 256))]:
            aps[nm] = nc.dram_tensor(nm, shp, BF16, kind="ExternalInput").ap()
        aps["ident"] = nc.dram_tensor("ident", (128, 128), BF16,
                                      kind="ExternalInput").ap()
        aps["wo"] = nc.dram_tensor("wo", (256, D), BF16, kind="ExternalInput").ap()
        for nm, shp in [("bqT", (128, 2)), ("bkT", (128, 2))]:
            aps[nm] = nc.dram_tensor(nm, shp, F32, kind="ExternalInput").ap()
        aps["out"] = nc.dram_tensor("out", (S, D), BF16,
                                    kind="ExternalOutput").ap()
        _emit(nc, aps)
        nc.compile()
        _NC = nc
    return _NC


def _run(inputs, trace=False):
    nc = _get_nc()
    f = np.float32
    bf = ml_dtypes.bfloat16
    q = np.asarray(inputs["query"], dtype=f)
    k = np.asarray(inputs["key"], dtype=f)
    v = np.asarray(inputs["value"], dtype=f)
    Wq = np.asarray(inputs["Wq"], dtype=f)
    Wk = np.asarray(inputs["Wk"], dtype=f)
    Wv = np.asarray(inputs["Wv"], dtype=f)
    Wo = np.asarray(inputs["Wo"], dtype=f)
    bq = np.asarray(inputs["bq"], dtype=f)
    bk = np.asarray(inputs["bk"], dtype=f)
    bv = np.asarray(inputs["bv"], dtype=f)
    bo = np.asarray(inputs["bo"], dtype=f)

    ident = np.eye(128, dtype=f)
    xT = {b: (np.ascontiguousarray(q[b].T).astype(bf),
              np.ascontiguousarray(k[b].T).astype(bf),
              np.ascontiguousarray(v[b].T).astype(bf)) for b in range(B)}
    def pack_w(W):
        # (1024, 256) -> (128, 2048) laid out [p, cc*1024 + d*128 + c]
        Wr = W.reshape(8, 128, 2, 128)
        return np.ascontiguousarray(
            Wr.transpose(1, 2, 0, 3).reshape(128, 2048)).astype(bf)

    in_maps = []
    for i in range(8):
        b, hg = divmod(i, 4)
        c0 = hg * 256
        in_maps.append({
            "xqT": xT[b][0], "xkT": xT[b][1], "xvT": xT[b][2],
            "wq": pack_w(Wq[:, c0:c0 + 256]),
            "wk": pack_w(Wk[:, c0:c0 + 256]),
            "wv": np.ascontiguousarray(Wv[:, c0:c0 + 256]).astype(bf),
            "bqT": np.ascontiguousarray(bq[c0:c0 + 256].reshape(2, 128).T),
            "bkT": np.ascontiguousarray(bk[c0:c0 + 256].reshape(2, 128).T),
            "wo": np.ascontiguousarray(Wo[c0:c0 + 256, :]).astype(bf),
            "ident": ident.astype(bf),
        })

    res = bass_utils.run_bass_kernel_spmd(nc, in_maps, core_ids=list(range(8)),
                                          trace=trace)
    out = np.zeros((B, S, D), dtype=f)
    for i in range(8):
        out[i // 4] += np.asarray(res.results[i]["out"], dtype=f)
    out += (bv @ Wo + bo)[None, None, :]
    return out, res


def kernel(**inputs):
    out, _ = _run(inputs, trace=False)
    return out

